# revision 5
# baseline (speedup 1.0000x reference)
"""Trainium2 Bass kernel for nn_AttLayer (4-head attention, softmax over queries).

Sharding: data-parallel over batch. 8 batch elements -> 8 NeuronCores, zero
collectives. Each core computes one batch element entirely in "transposed"
layout (channels/features on partitions, sequence on the free dim), which
makes every DMA layout-natural:

  x_b            : [64, 1024]  (natural layout of x[b] = [64, 32, 32])
  projT          : [7680, 1024] = W_aug^T @ x_aug   (bias via ones-row trick)
  scoresT[j, i]  : kT^T(d-tiles) @ qT               (j on partitions)
  softmax over i : free-dim reduction; exp+rowsum fused in one ACT op
  attT[d, i]     : v^T(j-tiles) @ exp               (1/denom folded into v)
  outT [64,1024] : W_out^T @ attT + b_out + x_b     (accumulated in PSUM)

All matmuls in bf16 (f32 PSUM accumulate).
"""

import numpy as np
import ml_dtypes

import concourse.bass as bass
import concourse.tile as tile
from concourse import bacc, mybir
from concourse.bass_utils import run_bass_kernel_spmd

NH = 4          # heads
D = 640         # per-head dim
C = 64          # channels
SEQ = 1024      # 32*32
SCALE = float(D) ** -0.5
N_CORES = 8
FP = mybir.dt.float32
BF = mybir.dt.bfloat16

JT = SEQ // 128     # 8 j-tiles (128 keys each)
DT = D // 128       # 5 d-tiles per head
IC = SEQ // 512     # 2 i-chunks (512 queries each)
KT_OUT = NH * D // 128  # 20 contraction tiles for the output projection
HPA3 = 3 * NH * D   # 7680

AF = mybir.ActivationFunctionType
ALU = mybir.AluOpType


def _build():
    nc = bacc.Bacc(None, target_bir_lowering=False)
    xa = nc.declare_dram_parameter("xa", [C + 1, SEQ], BF, isOutput=False)
    xf = nc.declare_dram_parameter("xf", [C, SEQ], FP, isOutput=False)
    wa = nc.declare_dram_parameter("wa", [C + 1, HPA3], BF, isOutput=False)
    wo = nc.declare_dram_parameter("wo", [128, KT_OUT, C], BF, isOutput=False)
    bo = nc.declare_dram_parameter("bo", [C, 1], FP, isOutput=False)
    out = nc.declare_dram_parameter("out", [C, SEQ], FP, isOutput=True)

    with tile.TileContext(nc) as tc:
        with (
            tc.tile_pool(name="consts", bufs=1) as consts,
            tc.tile_pool(name="hpool", bufs=2) as hpool,
            tc.tile_pool(name="pr", bufs=3, space="PSUM") as pr_psum,
            tc.tile_pool(name="sc", bufs=3, space="PSUM") as sc_psum,
            tc.tile_pool(name="o2", bufs=1, space="PSUM") as o2_psum,
        ):
            xa_sb = consts.tile([C + 1, SEQ], BF)
            nc.sync.dma_start(out=xa_sb[:], in_=xa[:, :])
            xf_sb = consts.tile([C, SEQ], FP)
            nc.sync.dma_start(out=xf_sb[:], in_=xf[:, :])
            wa_sb = consts.tile([C + 1, HPA3], BF)
            for h in range(NH):  # per-head chunks so head 0 can start early
                nc.sync.dma_start(
                    out=wa_sb[:, h * 3 * D:(h + 1) * 3 * D],
                    in_=wa[:, h * 3 * D:(h + 1) * 3 * D],
                )
            wo_sb = consts.tile([128, KT_OUT, C], BF)
            nc.sync.dma_start(out=wo_sb[:], in_=wo[:, :, :])
            bo_sb = consts.tile([C, 1], FP)
            nc.sync.dma_start(out=bo_sb[:], in_=bo[:, :])
            out_sb = consts.tile([C, SEQ], FP)

            # output-projection accumulators, live across all heads
            o2_tiles = [
                o2_psum.tile([C, 512], FP, tag=f"o2_{ic}", name=f"o2_{ic}")
                for ic in range(IC)
            ]

            for h in range(NH):
                qcol = h * 3 * D
                kcol = qcol + D
                vcol = qcol + 2 * D

                qT = hpool.tile([128, DT, SEQ], BF, tag="qT")
                kT = hpool.tile([128, DT, SEQ], BF, tag="kT")
                vs = hpool.tile([128, JT, D], BF, tag="vs")
                es = hpool.tile([128, JT, SEQ], BF, tag="es")
                aT = hpool.tile([128, DT, SEQ], BF, tag="aT")
                dpart = hpool.tile([128, JT, IC], FP, tag="dpart")
                den = hpool.tile([128, JT], FP, tag="den")
                rec = hpool.tile([128, JT], FP, tag="rec")

                # ---- q^T / k^T projections: psum[wcol-tile, i] = wa^T @ xa
                for col0, dst, eng in ((qcol, qT, "v"), (kcol, kT, "v")):
                    for d0 in range(DT):
                        for ic in range(IC):
                            ps = pr_psum.tile([128, 512], FP, tag="pr")
                            nc.tensor.matmul(
                                ps[:],
                                lhsT=wa_sb[:, col0 + d0 * 128: col0 + (d0 + 1) * 128],
                                rhs=xa_sb[:, ic * 512:(ic + 1) * 512],
                                start=True, stop=True,
                            )
                            dst_ap = dst[:, d0, ic * 512:(ic + 1) * 512]
                            if eng == "v":
                                nc.vector.tensor_copy(out=dst_ap, in_=ps[:])
                            else:
                                nc.scalar.copy(out=dst_ap, in_=ps[:])

                # ---- scoresT + fused exp/rowsum per (j-tile, i-chunk)
                for jt in range(JT):
                    for ic in range(IC):
                        ps = sc_psum.tile([128, 512], FP, tag="sc")
                        for d0 in range(DT):
                            nc.tensor.matmul(
                                ps[:],
                                lhsT=kT[:, d0, jt * 128:(jt + 1) * 128],
                                rhs=qT[:, d0, ic * 512:(ic + 1) * 512],
                                start=(d0 == 0), stop=(d0 == DT - 1),
                            )
                        nc.scalar.activation(
                            out=es[:, jt, ic * 512:(ic + 1) * 512],
                            in_=ps[:],
                            func=AF.Exp,
                            scale=SCALE,
                            accum_out=dpart[:, jt, ic:ic + 1],
                        )

                # ---- softmax denominators -> reciprocals (per key j)
                nc.vector.tensor_reduce(
                    out=den[:, :], in_=dpart[:, :, :],
                    axis=mybir.AxisListType.X, op=ALU.add,
                )
                nc.vector.reciprocal(out=rec[:, :], in_=den[:, :])

                # ---- v projection, scaled by 1/denom on the way to bf16
                for jt in range(JT):
                    for vc0, vcw in ((0, 512), (512, 128)):
                        ps = pr_psum.tile([128, 512], FP, tag="pr")
                        nc.tensor.matmul(
                            ps[:, :vcw],
                            lhsT=xa_sb[:, jt * 128:(jt + 1) * 128],
                            rhs=wa_sb[:, vcol + vc0: vcol + vc0 + vcw],
                            start=True, stop=True,
                        )
                        nc.scalar.mul(
                            out=vs[:, jt, vc0:vc0 + vcw],
                            in_=ps[:, :vcw],
                            mul=rec[:, jt:jt + 1],
                        )

                # ---- attT[d, i] = sum_j v'[j, d] * exp[j, i]
                for d0 in range(DT):
                    for ic in range(IC):
                        ps = pr_psum.tile([128, 512], FP, tag="pr")
                        for jt in range(JT):
                            nc.tensor.matmul(
                                ps[:],
                                lhsT=vs[:, jt, d0 * 128:(d0 + 1) * 128],
                                rhs=es[:, jt, ic * 512:(ic + 1) * 512],
                                start=(jt == 0), stop=(jt == JT - 1),
                            )
                        nc.scalar.copy(out=aT[:, d0, ic * 512:(ic + 1) * 512], in_=ps[:])

                # ---- output projection: out2T += W_out^T @ attT
                for d0 in range(DT):
                    kt = h * DT + d0
                    for ic in range(IC):
                        nc.tensor.matmul(
                            o2_tiles[ic][:],
                            lhsT=wo_sb[:, kt, :],
                            rhs=aT[:, d0, ic * 512:(ic + 1) * 512],
                            start=(kt == 0), stop=(kt == KT_OUT - 1),
                            skip_group_check=True,
                        )

            # ---- epilogue: + b_out (per-partition) + residual x, DMA out
            for ic in range(IC):
                nc.scalar.activation(
                    out=out_sb[:, ic * 512:(ic + 1) * 512],
                    in_=o2_tiles[ic][:],
                    func=AF.Identity,
                    bias=bo_sb[:, 0:1],
                    scale=1.0,
                )
            nc.vector.tensor_add(out=out_sb[:], in0=out_sb[:], in1=xf_sb[:])
            nc.sync.dma_start(out=out[:, :], in_=out_sb[:])

    nc.compile()
    return nc


_CACHE: dict = {}


def _get_nc():
    if "nc" not in _CACHE:
        _CACHE["nc"] = _build()
    return _CACHE["nc"]


def _prep_in_maps(x, W_proj, b_proj, W_out, b_out):
    bf = ml_dtypes.bfloat16
    x = np.ascontiguousarray(np.asarray(x, dtype=np.float32))
    x2 = x.reshape(N_CORES, C, SEQ)
    xa_all = np.empty((N_CORES, C + 1, SEQ), dtype=bf)
    xa_all[:, :C, :] = x2.astype(bf)
    xa_all[:, C, :] = np.float32(1.0)

    wa = np.empty((C + 1, HPA3), dtype=bf)
    wa[:C] = np.asarray(W_proj, dtype=np.float32).astype(bf)
    wa[C] = np.asarray(b_proj, dtype=np.float32).astype(bf)

    wo = np.ascontiguousarray(
        np.asarray(W_out, dtype=np.float32).reshape(KT_OUT, 128, C)
        .transpose(1, 0, 2).astype(bf)
    )
    bo = np.ascontiguousarray(np.asarray(b_out, dtype=np.float32).reshape(C, 1))

    return [
        {
            "xa": np.ascontiguousarray(xa_all[i]),
            "xf": np.ascontiguousarray(x2[i]),
            "wa": wa,
            "wo": wo,
            "bo": bo,
        }
        for i in range(N_CORES)
    ]


def run(x, t, W_proj, b_proj, W_out, b_out, trace=False, **trace_kwargs):
    in_maps = _prep_in_maps(x, W_proj, b_proj, W_out, b_out)
    res = run_bass_kernel_spmd(
        _get_nc(), in_maps, core_ids=list(range(N_CORES)),
        trace=trace, **trace_kwargs,
    )
    out = np.stack([res.results[i]["out"] for i in range(N_CORES)])
    return out.reshape(N_CORES, C, 32, 32), res


def kernel(x, t=None, W_proj=None, b_proj=None, W_out=None, b_out=None):
    out, _ = run(x, t, W_proj, b_proj, W_out, b_out, trace=False)
    return out


# revision 9
# speedup vs baseline: 1.2251x; 1.2251x over previous
"""Trainium2 Bass kernel for nn_AttLayer (4-head attention, softmax over queries).

Sharding: data-parallel over batch. 8 batch elements -> 8 NeuronCores, zero
collectives. Each core computes one batch element entirely in "transposed"
layout (channels/features on partitions, sequence on the free dim), which
makes every DMA layout-natural:

  x_b            : [64, 1024]  (natural layout of x[b] = [64, 32, 32])
  projT          : [7680, 1024] = W_aug^T @ x_aug   (bias via ones-row trick)
  scoresT[j, i]  : kT^T(d-tiles) @ qT               (j on partitions)
  softmax over i : free-dim reduction; exp+rowsum fused in one ACT op
  attT[d, i]     : v^T(j-tiles) @ (256*exp/den)     (normalized weights in fp8)
  outT [64,1024] : W_out^T @ attT + b_out + x_b     (accumulated in PSUM)

q/k/v and the normalized attention weights are fp8e4 so the two big matmuls
(scores, att) run in DoubleRow perf mode; the out-projection stays bf16.
The normalized weights 256*exp[j,i]/den[j] are bounded by 256 (each exp term
is a summand of its own denominator), so fp8e4 never overflows.
"""

import numpy as np
import ml_dtypes

import concourse.bass as bass
import concourse.tile as tile
from concourse import bacc, mybir
from concourse.bass_utils import run_bass_kernel_spmd

NH = 4          # heads
D = 640         # per-head dim
C = 64          # channels
SEQ = 1024      # 32*32
SCALE = float(D) ** -0.5
N_CORES = 8
FP = mybir.dt.float32
BF = mybir.dt.bfloat16
F8 = mybir.dt.float8e4
ES_SCALE = 256.0  # keep normalized weights inside fp8e4 normal range

JT = SEQ // 128     # 8 j-tiles (128 keys each)
DT = D // 128       # 5 d-tiles per head
IC = SEQ // 512     # 2 i-chunks (512 queries each)
KT_OUT = NH * D // 128  # 20 contraction tiles for the output projection
HPA3 = 3 * NH * D   # 7680

AF = mybir.ActivationFunctionType
ALU = mybir.AluOpType
DR = mybir.MatmulPerfMode.DoubleRow


def _build():
    nc = bacc.Bacc(None, target_bir_lowering=False)
    xa = nc.declare_dram_parameter("xa", [C + 1, SEQ], BF, isOutput=False)
    xf = nc.declare_dram_parameter("xf", [C, SEQ], FP, isOutput=False)
    wa = nc.declare_dram_parameter("wa", [C + 1, HPA3], BF, isOutput=False)
    wo = nc.declare_dram_parameter("wo", [128, KT_OUT, C], BF, isOutput=False)
    bo = nc.declare_dram_parameter("bo", [C, 1], FP, isOutput=False)
    out = nc.declare_dram_parameter("out", [C, SEQ], FP, isOutput=True)

    with tile.TileContext(nc) as tc:
        with (
            tc.tile_pool(name="consts", bufs=1) as consts,
            tc.tile_pool(name="hpool", bufs=2) as hpool,
            tc.tile_pool(name="pr", bufs=3, space="PSUM") as pr_psum,
            tc.tile_pool(name="sc", bufs=3, space="PSUM") as sc_psum,
            tc.tile_pool(name="o2", bufs=1, space="PSUM") as o2_psum,
        ):
            xa_sb = consts.tile([C + 1, SEQ], BF)
            nc.sync.dma_start(out=xa_sb[:], in_=xa[:, :])
            xf_sb = consts.tile([C, SEQ], FP)
            nc.sync.dma_start(out=xf_sb[:], in_=xf[:, :])
            wa_sb = consts.tile([C + 1, HPA3], BF)
            for h in range(NH):  # per-head chunks so head 0 can start early
                nc.sync.dma_start(
                    out=wa_sb[:, h * 3 * D:(h + 1) * 3 * D],
                    in_=wa[:, h * 3 * D:(h + 1) * 3 * D],
                )
            wo_sb = consts.tile([128, KT_OUT, C], BF)
            nc.sync.dma_start(out=wo_sb[:], in_=wo[:, :, :])
            bo_sb = consts.tile([C, 1], FP)
            nc.sync.dma_start(out=bo_sb[:], in_=bo[:, :])
            out_sb = consts.tile([C, SEQ], FP)

            # output-projection accumulators, live across all heads
            o2_tiles = [
                o2_psum.tile([C, 512], FP, tag=f"o2_{ic}", name=f"o2_{ic}")
                for ic in range(IC)
            ]

            for h in range(NH):
                qcol = h * 3 * D
                kcol = qcol + D
                vcol = qcol + 2 * D

                qT = hpool.tile([128, DT, SEQ], F8, tag="qT")
                kT = hpool.tile([128, DT, SEQ], F8, tag="kT")
                vs = hpool.tile([128, JT, D], F8, tag="vs")
                esr = hpool.tile([128, JT, SEQ], BF, tag="esr")   # raw exp
                es = hpool.tile([128, JT, SEQ], F8, tag="es")     # 256*exp/den
                aT = hpool.tile([128, DT, SEQ], BF, tag="aT")
                dpart = hpool.tile([128, JT, IC], FP, tag="dpart")
                den = hpool.tile([128, JT], FP, tag="den")
                rec = hpool.tile([128, JT], FP, tag="rec")

                # ---- q^T / k^T projections: psum[wcol-tile, i] = wa^T @ xa
                for col0, dst, eng in ((qcol, qT, "s"), (kcol, kT, "v")):
                    for d0 in range(DT):
                        for ic in range(IC):
                            ps = pr_psum.tile([128, 512], FP, tag="pr")
                            nc.tensor.matmul(
                                ps[:],
                                lhsT=wa_sb[:, col0 + d0 * 128: col0 + (d0 + 1) * 128],
                                rhs=xa_sb[:, ic * 512:(ic + 1) * 512],
                                start=True, stop=True,
                            )
                            dst_ap = dst[:, d0, ic * 512:(ic + 1) * 512]
                            if eng == "v":
                                nc.vector.tensor_copy(out=dst_ap, in_=ps[:])
                            else:
                                nc.scalar.copy(out=dst_ap, in_=ps[:])

                # ---- v projection (plain fp8 copy; 1/den now lives in es)
                for jt in range(JT):
                    for vc0, vcw in ((0, 512), (512, 128)):
                        ps = pr_psum.tile([128, 512], FP, tag="pr")
                        nc.tensor.matmul(
                            ps[:, :vcw],
                            lhsT=xa_sb[:, jt * 128:(jt + 1) * 128],
                            rhs=wa_sb[:, vcol + vc0: vcol + vc0 + vcw],
                            start=True, stop=True,
                        )
                        nc.scalar.copy(out=vs[:, jt, vc0:vc0 + vcw], in_=ps[:, :vcw])

                # ---- scoresT + fused exp/rowsum, then per-j-tile normalize
                for jt in range(JT):
                    for ic in range(IC):
                        ps = sc_psum.tile([128, 512], FP, tag="sc")
                        for kk in (0, 2, 4):
                            if kk < 4:
                                nc.tensor.matmul(
                                    ps[:],
                                    lhsT=kT[:, kk:kk + 2, jt * 128:(jt + 1) * 128],
                                    rhs=qT[:, kk:kk + 2, ic * 512:(ic + 1) * 512],
                                    start=(kk == 0), stop=False,
                                    perf_mode=DR,
                                )
                            else:
                                nc.tensor.matmul(
                                    ps[:],
                                    lhsT=kT[:, 4, jt * 128:(jt + 1) * 128],
                                    rhs=qT[:, 4, ic * 512:(ic + 1) * 512],
                                    start=False, stop=True,
                                )
                        nc.scalar.activation(
                            out=esr[:, jt, ic * 512:(ic + 1) * 512],
                            in_=ps[:],
                            func=AF.Exp,
                            scale=SCALE,
                            accum_out=dpart[:, jt, ic:ic + 1],
                        )
                    # den_jt = (sum_i exp)/ES_SCALE; es = exp/den_jt in fp8
                    nc.vector.tensor_add(
                        out=den[:, jt:jt + 1],
                        in0=dpart[:, jt, 0:1], in1=dpart[:, jt, 1:2],
                    )
                    nc.vector.tensor_scalar_mul(
                        den[:, jt:jt + 1], den[:, jt:jt + 1], 1.0 / ES_SCALE,
                    )
                    nc.vector.reciprocal(out=rec[:, jt:jt + 1], in_=den[:, jt:jt + 1])
                    nc.vector.tensor_scalar(
                        out=es[:, jt, :],
                        in0=esr[:, jt, :],
                        scalar1=rec[:, jt:jt + 1],
                        scalar2=None,
                        op0=ALU.mult,
                    )

                # ---- attT[d, i] = sum_j v[j, d] * es[j, i], undo ES_SCALE
                for d0 in range(DT):
                    for ic in range(IC):
                        ps = pr_psum.tile([128, 512], FP, tag="pr")
                        for jp in range(0, JT, 2):
                            nc.tensor.matmul(
                                ps[:],
                                lhsT=vs[:, jp:jp + 2, d0 * 128:(d0 + 1) * 128],
                                rhs=es[:, jp:jp + 2, ic * 512:(ic + 1) * 512],
                                start=(jp == 0), stop=(jp == JT - 2),
                                perf_mode=DR,
                            )
                        nc.vector.tensor_scalar_mul(
                            aT[:, d0, ic * 512:(ic + 1) * 512], ps[:], 1.0 / ES_SCALE,
                        )

                # ---- output projection: out2T += W_out^T @ attT
                for d0 in range(DT):
                    kt = h * DT + d0
                    for ic in range(IC):
                        nc.tensor.matmul(
                            o2_tiles[ic][:],
                            lhsT=wo_sb[:, kt, :],
                            rhs=aT[:, d0, ic * 512:(ic + 1) * 512],
                            start=(kt == 0), stop=(kt == KT_OUT - 1),
                            skip_group_check=True,
                        )

            # ---- epilogue: + b_out (per-partition) + residual x, DMA out
            for ic in range(IC):
                nc.scalar.activation(
                    out=out_sb[:, ic * 512:(ic + 1) * 512],
                    in_=o2_tiles[ic][:],
                    func=AF.Identity,
                    bias=bo_sb[:, 0:1],
                    scale=1.0,
                )
            nc.vector.tensor_add(out=out_sb[:], in0=out_sb[:], in1=xf_sb[:])
            nc.sync.dma_start(out=out[:, :], in_=out_sb[:])

    nc.compile()
    return nc


_CACHE: dict = {}


def _get_nc():
    if "nc" not in _CACHE:
        _CACHE["nc"] = _build()
    return _CACHE["nc"]


def _prep_in_maps(x, W_proj, b_proj, W_out, b_out):
    bf = ml_dtypes.bfloat16
    x = np.ascontiguousarray(np.asarray(x, dtype=np.float32))
    x2 = x.reshape(N_CORES, C, SEQ)
    xa_all = np.empty((N_CORES, C + 1, SEQ), dtype=bf)
    xa_all[:, :C, :] = x2.astype(bf)
    xa_all[:, C, :] = np.float32(1.0)

    wa = np.empty((C + 1, HPA3), dtype=bf)
    wa[:C] = np.asarray(W_proj, dtype=np.float32).astype(bf)
    wa[C] = np.asarray(b_proj, dtype=np.float32).astype(bf)

    wo = np.ascontiguousarray(
        np.asarray(W_out, dtype=np.float32).reshape(KT_OUT, 128, C)
        .transpose(1, 0, 2).astype(bf)
    )
    bo = np.ascontiguousarray(np.asarray(b_out, dtype=np.float32).reshape(C, 1))

    return [
        {
            "xa": np.ascontiguousarray(xa_all[i]),
            "xf": np.ascontiguousarray(x2[i]),
            "wa": wa,
            "wo": wo,
            "bo": bo,
        }
        for i in range(N_CORES)
    ]


def run(x, t, W_proj, b_proj, W_out, b_out, trace=False, **trace_kwargs):
    in_maps = _prep_in_maps(x, W_proj, b_proj, W_out, b_out)
    res = run_bass_kernel_spmd(
        _get_nc(), in_maps, core_ids=list(range(N_CORES)),
        trace=trace, **trace_kwargs,
    )
    out = np.stack([res.results[i]["out"] for i in range(N_CORES)])
    return out.reshape(N_CORES, C, 32, 32), res


def kernel(x, t=None, W_proj=None, b_proj=None, W_out=None, b_out=None):
    out, _ = run(x, t, W_proj, b_proj, W_out, b_out, trace=False)
    return out
